# revision 7
# baseline (speedup 1.0000x reference)
"""Dark-channel loss kernel for Trainium2 (8 NeuronCores, batch-parallel).

reference: loss = mean(|MaxPool3d((3,35,35), stride 1, pad (0,17,17))(1 - img)|)
         = 1 - mean(minpool_{3ch,35x35}(img))        (img in [0,1))

The van Herk sliding-min runs in two hand-registered custom DVE ops
(segmented scans with a SUB_DIM_DONE reset step state), fusing the
combine into the backward pass and eliminating mask tensors entirely:

  SEGMIN_F_ANT:  out = minimum.accumulate(in0) reset at each 35-elem page
  SEGMIN_C_ANT:  out = min(segmented-scan(in0), in1)  -- bwd pass fused
                 with the van Herk combine (in1 = fwd scan shifted +34,
                 both streamed reversed so out lands forward)

Per-core shard: 4 images [3,512,512] fp32. Engine assignment per image:
  Pool : SWDGE desc-gen for the cast DMAs (fp32->bf16 in the DMA; every
         image loads in 6 channel-half pieces) + pad memsets
  DVE  : channel-min at HALF-image granularity (one wide m01 TT + one
         3D-strided wt TT per half, bf16 2x; Pool/ACT cannot do
         elementwise min on trn2, and half granularity matches the DMA
         piece arrival -- whole-image chmin measures 3us slower), then
         W: SEGMIN_F(wt) -> sW; SEGMIN_C(wt_rev, sW[34:]_rev) -> mw
         H: SEGMIN_F(ut) -> sH; SEGMIN_C(ut_rev, sH[34:]_rev) -> mhw
         images 0 and 3 run W half-granular (pipeline fill / drain)
  PE   : 16 transposes mw -> PSUM; ones-matmuls accumulate sum(mhw)
         into two PSUM groups (images 0..2 / image 3) so the first ACT
         reduce overlaps the last image's H pass
  ACT  : 4 PSUM->SBUF copies into the padded ut tile; 2 final reduces
Final: host sums the two per-core partials; loss = 1 - sum/(N*H*W).

Padded geometry (d=17, window 35, block 35): each 560-col segment holds
512 data cols at [17,529); combine output for pixel j is at col j; all
offsets stay 4B-aligned. bf16 is exact for mins under monotone rounding;
|loss error| <= 2^-9 * mean(min) ~ 5e-7 (measured rel err 6.05e-9).
Measured ~64-66us/exec (timeloop-delta slope; interleaved same-process
duels) vs ~90-98us for the previous kernel.
"""

import os
from dataclasses import dataclass, field

import numpy as np

N_CORES = 8
N, C, H, W = 32, 3, 512, 512
PER = N // N_CORES          # images per core
P = 128
WIN = 35
NB = 16                     # blocks per 560-segment
SEG = NB * WIN              # 560
D = 17                      # data offset within a segment
CH = 4                      # h-chunks / w-chunks per image
LW = CH * SEG               # 2240
NBT = CH * NB               # 64 pages of 35 per image tile
SLOP = 64                   # extra cols on scan tiles for the +34 read
BIG = 1e9

TIMELOOP = int(os.environ.get("DC_TIMELOOP", "0"))

_cached_nc = None
_SEG_COMPILE_CACHE = {}
_SEG_OPS = {}


def _seg_lower(spec, ver):
    """dve_spec.lower() with a SUB_DIM_DONE step state that RESETS each
    scan register (state = op(init, expr) consuming the boundary element)
    instead of the stock page-counter increment."""
    import concourse.dve_spec as ds

    n_lanes, n_stages = ds.N_LANES[ver], ds.N_STAGES[ver]
    ds._validate_body(spec, ver)
    spec = ds._hoist_stream_invariant_ops(spec)
    scans = ds._collect(spec.body, ds.Scan)
    assert scans and not ds._collect(spec.body, ds.Latch)
    placement = ds._build_placement(spec, scans, n_stages, n_lanes)
    seed_ov, _ = ds._scan_overrides(scans, placement.node_stage)
    step_ov = {}
    for sc in scans:
        d = placement.node_stage[sc]
        step_ov[d] = ds._Stage(sc.op, ds._scan_init(sc), sc.expr)
    body_lvs = ds._body_scan_leaves(spec)
    consume = (ds.Src0 in body_lvs, ds.Src1 in body_lvs)
    T = ds.Trigger
    states = [
        ds._State(placement=placement, overrides=seed_ov,
                  trigger=ds.COUNT_ONCE, repeat=1, next=(1, 0, 0),
                  write_out=False),
        ds._State(placement=placement, consume=consume,
                  trigger=(T.SRC_TENSOR_DONE, T.SUB_DIM_DONE, T.NONE),
                  next=(0, 2, 0)),
        ds._State(placement=placement, consume=consume, overrides=step_ov,
                  trigger=(T.SRC_TENSOR_DONE, T.SUB_DIM_DONE, T.COUNT),
                  next=(0, 2, 1), repeat=1),
    ]
    uops = [ds._assemble(s) for s in states]
    for u in uops:
        u.validate(ver)
    return uops


@dataclass(frozen=True)
class _SegDveOp:
    """Duck-typed stand-in for dve_ops.DveOp whose compile() runs the
    segmented-reset lowering above."""

    name: str
    spec: object
    subdim: bool = True
    perf_en: dict = field(default_factory=dict)
    uops_sha: dict = field(default_factory=dict)

    def compile(self, ver):
        from concourse.dve_uop import DveOpSpec
        import concourse.dve_ops as dops
        import concourse.dve_spec as ds

        key = (self.name, ver)
        if key not in _SEG_COMPILE_CACHE:
            _SEG_COMPILE_CACHE[key] = DveOpSpec(
                name=self.name,
                opcode=dops.get_dve_sub_opcode(self.name),
                uops=_seg_lower(self.spec, ver),
                rd1_en=ds.Src1 in ds.spec_leaves(self.spec),
            )
        return _SEG_COMPILE_CACHE[key]


def _register_dve_ops():
    """Register the two segmented-scan ops in dve_ops' registries (new rows
    appended after the stock OPS; idempotent)."""
    if _SEG_OPS:
        return _SEG_OPS
    import concourse.dve_ops as dops
    import concourse.dve_spec as ds
    from concourse.dve_spec import Spec, Src0, Src1, C0, minn, scan, AluOp

    def ref_f(in0, in1, c0, c1, c2):
        # segmented min-scan along the last axis of [P, S, N]
        return np.minimum.accumulate(np.asarray(in0, np.float32), axis=-1)

    def ref_c(in0, in1, c0, c1, c2):
        r = np.minimum.accumulate(np.asarray(in0, np.float32), axis=-1)
        y = np.asarray(in1, np.float32).reshape(r.shape)
        return np.minimum(r, y)

    f = _SegDveOp("SEGMIN_F_ANT",
                  Spec(body=scan(AluOp.MIN, Src0, init=C0), reference=ref_f))
    c = _SegDveOp("SEGMIN_C_ANT",
                  Spec(body=minn(scan(AluOp.MIN, Src0, init=C0), Src1),
                       reference=ref_c))
    for op in (f, c):
        if op.name not in {o.name for o in dops.OPS}:
            dops.OPS.append(op)
        dops.CUSTOM_DVE_SPECS[op.name] = op.spec
        dops._SUB_OPCODE_FOR_NAME[op.name] = (
            dops._CUSTOM_DVE_ROW_BASE
            + next(i for i, o in enumerate(dops.OPS) if o.name == op.name))
    _SEG_OPS.update(f=f, c=c)
    return _SEG_OPS


def _build_nc(timeloop=TIMELOOP):
    import concourse.bacc as bacc
    import concourse.mybir as mybir
    from concourse.tile import TileContext
    from concourse.masks import make_identity

    ops = _register_dve_ops()
    dt = mybir.dt
    Alu = mybir.AluOpType

    nc = bacc.Bacc("TRN2")
    img = nc.declare_dram_parameter("img", [PER, C, H, W], dt.float32,
                                    isOutput=False)
    out = nc.declare_dram_parameter("out", [1, 2], dt.float32, isOutput=True)

    with TileContext(nc) as tc:
        with (
            tc.tile_pool(name="consts", bufs=1) as consts,
            tc.tile_pool(name="big", bufs=3) as bigp,
            tc.tile_pool(name="mm", bufs=4) as mmp,
            tc.tile_pool(name="sr", bufs=2) as srp,
            tc.tile_pool(name="psT", bufs=6, space="PSUM") as psT,
            tc.tile_pool(name="psS", bufs=1, space="PSUM") as psS,
        ):
            ident = consts.tile([P, P], dt.bfloat16, tag="ident")
            ones = consts.tile([P, 1], dt.bfloat16, tag="ones")
            wring = [consts.tile([P, LW], dt.bfloat16, tag=f"wt{i}",
                                 name=f"wt{i}") for i in range(2)]
            uring = [consts.tile([P, LW], dt.bfloat16, tag=f"ut{i}",
                                 name=f"ut{i}") for i in range(2)]
            sring = [consts.tile([P, LW + SLOP], dt.bfloat16, tag=f"s{i}",
                                 name=f"s{i}") for i in range(4)]
            cs = psS.tile([1, 512], dt.float32, tag="csum")
            cs2 = psS.tile([1, 512], dt.float32, tag="csum2")

            def emit_consts():
                # const setup on Pool/PE only, emitted AFTER the first two
                # loads' desc-gen so image 0's DMA is in flight first; lives
                # inside the timeloop so per-iter time matches a single shot
                make_identity(nc, ident[:])
                nc.gpsimd.memset(ones[:], 1.0)
                for t in wring + uring:
                    t3 = t[:].rearrange("p (nb l) -> p nb l", l=SEG)
                    nc.gpsimd.memset(t3[:, :, 0:D], 1.0)
                    nc.gpsimd.memset(t3[:, :, D + 512:SEG], 1.0)
                for t in sring:
                    nc.gpsimd.memset(t[:, LW:LW + SLOP], 1.0)

            import contextlib
            loop_ctx = (tc.For_i(0, timeloop, 1) if timeloop
                        else contextlib.nullcontext())
            with loop_ctx:
                sink1 = consts.tile([1, 512], dt.float32, tag="sink1")

                def emit_sink_a(tot2=None):
                    nc.scalar.activation(
                        out=sink1[:], in_=cs[:],
                        func=mybir.ActivationFunctionType.Copy,
                        accum_out=tot2[:, 0:1])

                tot2 = consts.tile([1, 2], dt.float32, tag="tot2")
                _body_emit(nc, tc, mybir, Alu, dt, img, bigp, mmp, srp,
                           psT, ident, ones, wring, uring, sring,
                           (cs, cs2, lambda: emit_sink_a(tot2)), ops,
                           emit_consts)

            sink2 = consts.tile([1, 512], dt.float32, tag="sink2")
            tot2 = consts.tile([1, 2], dt.float32, tag="tot2")
            nc.scalar.activation(out=sink2[:], in_=cs2[:],
                                 func=mybir.ActivationFunctionType.Copy,
                                 accum_out=tot2[:, 1:2])
            nc.scalar.dma_start(out=out[:], in_=tot2[:])

    nc.compile()
    return nc


def _body_emit(nc, tc, mybir, Alu, dt, img, bigp, mmp, srp, psT,
               ident, ones, wring, uring, sring, cs_pack, ops, emit_consts):
    cs, cs2, emit_sink_a = cs_pack
    n_mm_a = (PER - 1) * CH   # images 0..PER-2 accumulate into cs
    n_mm_b = CH               # last image into cs2
    mm_i = 0
    OPF, OPC = ops["f"], ops["c"]

    def segmin_f(s, src, npages=NBT):
        """s[:L] = segmented fwd min-scan of src (pages of 35)."""
        L = npages * WIN
        nc.vector._custom_dve(
            OPF, out=s[:, 0:L].rearrange("p (nb w) -> p nb w", w=WIN),
            in0=src.rearrange("p (nb w) -> p nb w", w=WIN), s0=BIG)

    def segmin_c(dst, src, s, npages=NBT):
        """dst = min(segmented bwd min-scan of src, s shifted +34) —
        streamed reversed so dst lands in forward order."""
        L = npages * WIN
        nc.vector._custom_dve(
            OPC,
            out=dst[:, ::-1].rearrange("p (nb w) -> p nb w", w=WIN),
            in0=src[:, ::-1].rearrange("p (nb w) -> p nb w", w=WIN),
            in1=s[:, 34:L + 34][:, ::-1],
            s0=BIG)

    def load(n, chunked=False):
        # SWDGE cast-DMA (fp32->bf16 in the DMA; only SWDGE casts). Split
        # ch01/ch2 so m01 can start at 2/3 of the image's DMA; image 0 is
        # chunk-split further to shorten pipeline fill.
        big = bigp.tile([P, 3 * 2048], dt.bfloat16, tag="big", name=f"big_{n}")
        b4 = big[:].rearrange("p (k c w) -> p k c w", k=3, c=CH)
        i4 = img[n].rearrange("k (c p) w -> p k c w", p=P)
        if chunked:
            # 6 pieces (channel x image-half): the DMA AP balancer caps at
            # 3 dims, so half-image pieces must be single-channel; first
            # half lands in ~3 pieces so chmin/scans start ~3us in
            for cc in (slice(0, 2), slice(2, 4)):
                for k in range(3):
                    nc.gpsimd.dma_start(out=b4[:, k, cc], in_=i4[:, k, cc])
        else:
            nc.gpsimd.dma_start(out=b4[:, 0:2], in_=i4[:, 0:2])
            nc.gpsimd.dma_start(out=b4[:, 2:3], in_=i4[:, 2:3])
        return big

    def chmin(n, big, chunks=None):
        # channel min on DVE (bf16 2x mode); optionally per h-chunk
        wt = wring[n % 2]
        wt3 = wt[:].rearrange("p (nb l) -> p nb l", l=SEG)
        if chunks is None:
            m01 = mmp.tile([P, 2048], dt.bfloat16, tag="m01",
                           name=f"m01_{n}")
            nc.vector.tensor_tensor(out=m01[:], in0=big[:, 0:2048],
                                    in1=big[:, 2048:4096], op=Alu.min)
            m3 = m01[:].rearrange("p (nb l) -> p nb l", l=512)
            b3 = big[:, 4096:6144].rearrange("p (nb l) -> p nb l", l=512)
            nc.vector.tensor_tensor(out=wt3[:, :, D:D + 512], in0=m3,
                                    in1=b3, op=Alu.min)
            return wt
        # contiguous chunk range -> one wide m01 TT + one 3D-strided wt TT
        # (fewer instruction boundaries than per-chunk pairs, same dataflow)
        c0, ln = chunks[0], len(chunks)
        assert list(chunks) == list(range(c0, c0 + ln))
        wd = 512 * ln
        m01 = mmp.tile([P, wd], dt.bfloat16, tag=f"m01r{ln}",
                       name=f"m01_{n}_{c0}")
        nc.vector.tensor_tensor(
            out=m01[:], in0=big[:, 512 * c0:512 * c0 + wd],
            in1=big[:, 2048 + 512 * c0:2048 + 512 * c0 + wd], op=Alu.min)
        m3v = m01[:].rearrange("p (c l) -> p c l", l=512)
        b3v = big[:, 4096 + 512 * c0:4096 + 512 * c0 + wd].rearrange(
            "p (c l) -> p c l", l=512)
        nc.vector.tensor_tensor(out=wt3[:, c0:c0 + ln, D:D + 512],
                                in0=m3v, in1=b3v, op=Alu.min)
        return wt

    def w_pass(n, wt):
        # whole-image: one fwd + one fused bwd/combine scan over [P,64,35]
        sW = sring[2 * (n % 2)]
        mw = mmp.tile([P, LW], dt.bfloat16, tag="mw", name=f"mw_{n}")
        segmin_f(sW, wt[:])
        segmin_c(mw[:], wt[:], sW[:])
        return mw

    def w0_pass(n, big):
        # image 0 (pipeline fill): chunk-granular chmin + scans, ordered
        # f(c+1) before c(c): c(c)'s +34 read crosses into chunk c+1's
        # fwd-scan region (only at never-consumed outputs, but it must
        # hold written, finite values)
        # half-granular (the DMA pieces deliver channel-halves, so chunk
        # granularity gains no earlier start -- halves save op overhead)
        wt = chmin(n, big, chunks=[0, 1])
        sW = sring[2 * (n % 2)]
        mw = mmp.tile([P, LW], dt.bfloat16, tag="mw", name=f"mw_{n}")
        hw2 = 2 * SEG
        segmin_f(sW[:, 0:hw2 + SLOP], wt[:, 0:hw2], npages=2 * NB)
        chmin(n, big, chunks=[2, 3])
        segmin_f(sW[:, hw2:2 * hw2 + SLOP], wt[:, hw2:2 * hw2],
                 npages=2 * NB)
        segmin_c(mw[:, 0:hw2], wt[:, 0:hw2], sW[:, 0:hw2 + SLOP],
                 npages=2 * NB)
        segmin_c(mw[:, hw2:2 * hw2], wt[:, hw2:2 * hw2],
                 sW[:, hw2:2 * hw2 + SLOP], npages=2 * NB)
        return mw

    def trans(n, mw):
        # mw col j of segment c is output pixel j; transpose the 4 aligned
        # 128-col groups per segment into PSUM, ACT-stage into padded ut.
        # c-outer: chunk c's transposes run as soon as its W pass lands;
        # the ACT copies then drain back-to-back after chunk CH-1.
        ut = uring[n % 2]
        ut3 = ut[:].rearrange("p (nb l) -> p nb l", l=SEG)
        m3 = mw[:].rearrange("p (nb l) -> p nb l", l=SEG)
        pts = [psT.tile([P, 512], dt.bfloat16, tag="pt", name=f"pt_{n}_{j}")
               for j in range(CH)]
        for c in range(CH):
            for j in range(CH):
                nc.tensor.transpose(pts[j][:, P * c:P * (c + 1)],
                                    m3[:, c, P * j:P * (j + 1)],
                                    ident[:])
        for j in range(CH):
            nc.scalar.copy(out=ut3[:, j, D:D + 512], in_=pts[j][:])
        return ut

    def w3_trans_pass(n, wt):
        """last image: per-chunk W scans with that chunk's 4 transposes
        emitted immediately, then the 4 ACT copies — so ut staging (and the
        final H) starts as soon as the last chunk's combine lands."""
        sW = sring[2 * (n % 2)]
        mw = mmp.tile([P, LW], dt.bfloat16, tag="mw", name=f"mw_{n}")
        w3 = wt[:].rearrange("p (nb l) -> p nb l", l=SEG)
        m3 = mw[:].rearrange("p (nb l) -> p nb l", l=SEG)
        ut = uring[n % 2]
        ut3 = ut[:].rearrange("p (nb l) -> p nb l", l=SEG)
        pts = [psT.tile([P, 512], dt.bfloat16, tag="pt", name=f"pt_{n}_{j}")
               for j in range(CH)]

        def emit_trans(c):
            for j in range(CH):
                nc.tensor.transpose(pts[j][:, P * c:P * (c + 1)],
                                    m3[:, c, P * j:P * (j + 1)], ident[:])

        hw2 = 2 * SEG
        segmin_f(sW[:, 0:hw2 + SLOP], wt[:, 0:hw2], npages=2 * NB)
        segmin_f(sW[:, hw2:2 * hw2 + SLOP], wt[:, hw2:2 * hw2],
                 npages=2 * NB)
        segmin_c(mw[:, 0:hw2], wt[:, 0:hw2], sW[:, 0:hw2 + SLOP],
                 npages=2 * NB)
        emit_trans(0)
        emit_trans(1)
        segmin_c(mw[:, hw2:2 * hw2], wt[:, hw2:2 * hw2],
                 sW[:, hw2:2 * hw2 + SLOP], npages=2 * NB)
        emit_trans(2)
        emit_trans(3)
        for j in range(CH):
            nc.scalar.copy(out=ut3[:, j, D:D + 512], in_=pts[j][:])
        return ut

    def h_pass(n, ut, per_chunk=False):
        nonlocal mm_i
        sH = sring[2 * (n % 2) + 1]
        if not per_chunk:
            segmin_f(sH, ut[:])
            mhw = mmp.tile([P, LW], dt.bfloat16, tag="mhw", name=f"mhw_{n}")
            segmin_c(mhw[:], ut[:], sH[:])
            h3 = mhw[:].rearrange("p (nb l) -> p nb l", l=SEG)
            for j in range(CH):
                nc.tensor.matmul(cs[:], ones[:], h3[:, j, 0:512],
                                 start=(mm_i == 0),
                                 stop=(mm_i == n_mm_a - 1),
                                 skip_group_check=True)
                mm_i += 1
            return
        # last image: per-chunk so H starts as each ACT copy lands
        ut3 = ut[:].rearrange("p (nb l) -> p nb l", l=SEG)
        for j in range(CH):
            seg = ut3[:, j, :]
            sseg = sH[:, j * SEG:(j + 1) * SEG + SLOP]
            segmin_f(sseg, seg, npages=NB)
            mhw = mmp.tile([P, SEG], dt.bfloat16, tag="mhw_c",
                           name=f"mhw_c_{n}_{j}")
            segmin_c(mhw[:], seg, sseg, npages=NB)
            nc.tensor.matmul(cs2[:], ones[:], mhw[:, 0:512],
                             start=(j == 0), stop=(j == CH - 1),
                             skip_group_check=True)

    # software pipeline; DVE program order per image n:
    #   chmin(n+1), W(n), H(n-1) — so chmin stays one image ahead and
    #   transposes/staging of n overlap H(n-1) + W(n+1)
    bigs, wts, uts, mws = {}, {}, {}, {}
    bigs[0] = load(0, chunked=True)
    bigs[1] = load(1)
    emit_consts()
    for n in range(PER):
        if n + 2 < PER:
            bigs[n + 2] = load(n + 2)
        if n == 0:
            mws[0] = w0_pass(0, bigs.pop(0))
        elif n == PER - 1:
            uts[n] = w3_trans_pass(n, wts.pop(n))
        else:
            mws[n] = w_pass(n, wts.pop(n))
        if n + 1 < PER:
            wts[n + 1] = chmin(n + 1, bigs.pop(n + 1))
        if n - 1 in uts:
            h_pass(n - 1, uts.pop(n - 1))
        if n in mws:
            uts[n] = trans(n, mws.pop(n))
        if n == PER - 1:
            emit_sink_a()
    h_pass(PER - 1, uts.pop(PER - 1), per_chunk=True)


def _get_nc():
    global _cached_nc
    if _cached_nc is None:
        _cached_nc = _build_nc()
    return _cached_nc


def _finish(results):
    partials = np.array([float(np.sum(results[i]["out"], dtype=np.float64))
                         for i in range(N_CORES)])
    loss = 1.0 - float(np.sum(partials, dtype=np.float64)) / (N * H * W)
    return np.asarray(loss, dtype=np.float32)


def kernel(img):
    from concourse.bass_utils import run_bass_kernel_spmd
    img_np = np.asarray(img, dtype=np.float32)
    assert img_np.shape == (N, C, H, W), img_np.shape
    shards = img_np.reshape(N_CORES, PER, C, H, W)
    in_maps = [{"img": np.ascontiguousarray(shards[i])}
               for i in range(N_CORES)]
    res = run_bass_kernel_spmd(_get_nc(), in_maps, list(range(N_CORES)))
    return _finish(res.results)
